# revision 1
# baseline (speedup 1.0000x reference)
"""Multi-head attention forward, sharded head-parallel across 8 NeuronCores.

Per core c (heads 2c, 2c+1):
  qT/kT/vT = (x @ W{q,k,v}_c.T).T        computed as W.T-tiled matmuls vs xT
  scoresT  = kT_chunk.T @ qT             [k-pos partitions, q-pos free]
  probsT   = exp(scoresT) (*causal mask via affine_select)
  av+rowsum: out.T = [v | 1].T @ probsT  (ones column yields softmax denom)
  normalize by PE-broadcast reciprocal, then out_projT partial
Host: sum the 8 partial [1024, 4096] outputs, transpose, add bias.
"""
import sys

sys.path.insert(0, "/opt/trn_rl_repo")

import ml_dtypes
import numpy as np

BF16 = ml_dtypes.bfloat16

B, S, D = 2, 2048, 1024
H, HD = 16, 64
NCORES = 8
SEC = 128           # output dims per core per section (2 heads * 64)
BS = B * S          # 4096
NT = BS // 512      # 8 seq tiles of 512
EC = D // 128       # 8 embed chunks
QT = S // 512       # 4 q-tiles per (b,h)
KC = S // 128       # 16 k-chunks per (b,h)

_cache = {}


def _build(mask_mode):
    import concourse.bass as bass
    import concourse.tile as tile
    from concourse import bacc, mybir

    f32 = mybir.dt.float32
    bf16 = mybir.dt.bfloat16
    Exp = mybir.ActivationFunctionType.Exp

    nc = bacc.Bacc("TRN2", target_bir_lowering=False, debug=False,
                   num_devices=NCORES)

    xT = nc.dram_tensor("xT", [D, BS], bf16, kind="ExternalInput")
    wqkvT = nc.dram_tensor("wqkvT", [D, 3 * SEC], bf16, kind="ExternalInput")
    woT = nc.dram_tensor("woT", [SEC, D], bf16, kind="ExternalInput")
    # consts: [:, 0:64] = eye(64) on both partition halves; [:, 64:160] = 1.0
    consts = nc.dram_tensor("consts", [128, 160], bf16, kind="ExternalInput")
    if mask_mode == "general":
        maskT = nc.dram_tensor("maskT", [S, S], bf16, kind="ExternalInput")
    out_pT = nc.dram_tensor("out_pT", [D, BS], f32, kind="ExternalOutput")

    with tile.TileContext(nc) as tc:
        with (
            nc.allow_low_precision(reason="fp32r passthrough transpose"),
            tc.tile_pool(name="singles", bufs=1) as singles,
            tc.tile_pool(name="qkv", bufs=1) as qkv,
            tc.tile_pool(name="xp", bufs=4) as xp,
            tc.tile_pool(name="v1p", bufs=1) as v1p,
            tc.tile_pool(name="pp", bufs=4) as pp,
            tc.tile_pool(name="np_", bufs=2) as np_,
            tc.tile_pool(name="fo", bufs=2) as fo,
        ):
            w_sb = singles.tile([128, EC, 3 * SEC], bf16)
            wqr = wqkvT.rearrange("(ec p) c -> p ec c", p=128)
            for ec in range(EC):
                nc.sync.dma_start(out=w_sb[:, ec, :], in_=wqr[:, ec, :])
            woT_sb = singles.tile([128, D], bf16)
            nc.gpsimd.dma_start(out=woT_sb[:], in_=woT[:])
            ident64 = singles.tile([128, 64], bf16)
            nc.gpsimd.dma_start(out=ident64[:], in_=consts[:, 0:64])
            ones1 = singles.tile([1, 64], f32)
            nc.vector.memset(ones1[:], 1.0)

            xfull = qkv.tile([128, EC, BS], bf16)
            xTr = xT.rearrange("(ec p) s -> p ec s", p=128)
            nc.sync.dma_start(out=xfull[:, 0, 0:1024], in_=xTr[:, 0, 0:1024])
            nc.sync.dma_start(out=xfull[:, 0, 1024:BS],
                              in_=xTr[:, 0, 1024:BS])
            for ec in range(1, EC):
                nc.sync.dma_start(out=xfull[:, ec, :], in_=xTr[:, ec, :])
            qT = qkv.tile([128, BS], bf16)
            kT = qkv.tile([128, BS], bf16)
            vT = qkv.tile([128, BS], bf16)
            ocat = qkv.tile([128, BS], bf16)

            # ---- stage A: qkvT projections ----
            with tc.tile_pool(name="psA", bufs=2, space="PSUM") as psA:
                for n in range(NT):
                    pq = psA.tile([128, 512], f32, tag="pq")
                    pk = psA.tile([128, 512], f32, tag="pk")
                    pv = psA.tile([128, 512], f32, tag="pv")
                    for ec in range(EC):
                        xs = xfull[:, ec, 512 * n:512 * (n + 1)]
                        st, sp = ec == 0, ec == EC - 1
                        nc.tensor.matmul(pq[:], w_sb[:, ec, 0:128],
                                         xs, start=st, stop=sp)
                        nc.tensor.matmul(pk[:], w_sb[:, ec, 128:256],
                                         xs, start=st, stop=sp)
                        nc.tensor.matmul(pv[:], w_sb[:, ec, 256:384],
                                         xs, start=st, stop=sp)
                    sl = slice(512 * n, 512 * (n + 1))
                    nc.vector.tensor_copy(qT[:, sl], pq[:])
                    nc.vector.tensor_copy(kT[:, sl], pk[:])
                    nc.vector.tensor_copy(vT[:, sl], pv[:])

            # ---- stage B: attention, both heads paired per b ----
            # scores for the two heads run concurrently on PE row-groups
            # 0-63 / 64-127 (K=64 each); av is K=128 full-array per head.
            with (
                tc.tile_pool(name="psS", bufs=3, space="PSUM") as psS,
                tc.tile_pool(name="psO", bufs=2, space="PSUM") as psO,
            ):
                v1bs = []
                for b in range(B):
                    base = S * b
                    v1s = []
                    for lh in range(2):
                        v1 = v1p.tile([128, KC, HD + 1], bf16,
                                      tag=f"v1{b}{lh}")
                        nc.sync.dma_start(out=v1[:, :, HD],
                                          in_=consts[:, 64:64 + KC])
                        v1s.append(v1)
                    for i in range(KC):
                        for lh in range(2):
                            hsl = slice(64 * lh, 64 * (lh + 1))
                            pt = psS.tile([128, 64], bf16, tag="ps")
                            nc.tensor.transpose(
                                pt[:],
                                vT[hsl, base + 128 * i:base + 128 * (i + 1)],
                                ident64[hsl, :])
                            nc.vector.tensor_copy(v1s[lh][:, i, 0:HD], pt[:])
                    v1bs.append(v1s)
                for b in range(B):
                    base = S * b
                    v1s = v1bs[b]
                    norm_tiles = []
                    for t in range(QT):
                        qsl = slice(base + 512 * t, base + 512 * (t + 1))
                        njc = 4 * t + 4 if mask_mode == "causal" else KC
                        pos = [psO.tile([HD + 1, 512], f32, tag="po",
                                        name=f"po{_lh}")
                               for _lh in range(2)]
                        # j descending: masked (diag) groups first, so their
                        # gpsimd mask ops overlap later unmasked groups' PE
                        for ji, j in enumerate(reversed(range(njc))):
                            ksl = slice(base + 128 * j, base + 128 * (j + 1))
                            ps = psS.tile([128, 1024], f32, tag="ps")
                            for lh in range(2):
                                hsl = slice(64 * lh, 64 * (lh + 1))
                                nc.tensor.matmul(
                                    ps[:, 512 * lh:512 * (lh + 1)],
                                    kT[hsl, ksl], qT[hsl, qsl],
                                    start=True, stop=True)
                            pr = pp.tile([128, 1024], bf16, tag="pr")
                            nc.scalar.activation(pr[:], ps[:], Exp)
                            if mask_mode == "causal" and j >= 4 * t:
                                jm = j - 4 * t
                                prv = pr.rearrange("p (lh c) -> p lh c", lh=2)
                                nc.gpsimd.affine_select(
                                    out=prv, in_=prv,
                                    compare_op=mybir.AluOpType.is_ge,
                                    fill=0.0, base=-128 * jm,
                                    channel_multiplier=-1,
                                    pattern=[[0, 2], [1, 512]])
                            elif mask_mode == "general":
                                msk = xp.tile([128, 512], bf16, tag="msk")
                                nc.sync.dma_start(
                                    out=msk[:],
                                    in_=maskT[128 * j:128 * (j + 1),
                                              512 * t:512 * (t + 1)])
                                for lh in range(2):
                                    sub = pr[:, 512 * lh:512 * (lh + 1)]
                                    nc.vector.tensor_mul(sub, sub, msk[:])
                            for lh in range(2):
                                nc.tensor.matmul(
                                    pos[lh][:], v1s[lh][:, j, :],
                                    pr[:, 512 * lh:512 * (lh + 1)],
                                    start=(ji == 0), stop=(ji == njc - 1))
                        for lh in range(2):
                            ol = np_.tile([HD, 512], f32, tag="ol", bufs=8,
                                          name=f"ol{t}{lh}")
                            nc.vector.tensor_copy(ol[:], pos[lh][0:HD, :])
                            lc = np_.tile([1, 512], f32, tag="lc")
                            nc.vector.tensor_copy(lc[:],
                                                  pos[lh][HD:HD + 1, :])
                            rc = np_.tile([1, 512], f32, tag="rc", bufs=8,
                                          name=f"rc{t}{lh}")
                            nc.vector.reciprocal_approx_fast(rc[:], lc[:])
                            norm_tiles.append((t, lh, ol, rc))
                    # deferred normalization: PE bcasts bunched, no
                    # head-of-line stall inside the t loop
                    for t, lh, ol, rc in norm_tiles:
                        hsl = slice(64 * lh, 64 * (lh + 1))
                        qsl = slice(base + 512 * t, base + 512 * (t + 1))
                        pb = psS.tile([64, 512], f32, tag="ps")
                        nc.tensor.matmul(pb[:], ones1[:], rc[:],
                                         start=True, stop=True)
                        bs_ = np_.tile([64, 512], bf16, tag="bs")
                        nc.scalar.copy(bs_[:], pb[:])
                        nc.gpsimd.tensor_mul(ocat[hsl, qsl], ol[:], bs_[:])

            # ---- stage C: out_projT partial ----
            # n-outer so n-tiles of batch 0 can overlap batch 1 attention;
            # one batched store per n-tile across all oc chunks.
            out_r = out_pT.rearrange("(oc p) s -> p oc s", p=128)
            with tc.tile_pool(name="psF", bufs=4, space="PSUM") as psF:
                for n in range(NT):
                    ssl = slice(512 * n, 512 * (n + 1))
                    ft = fo.tile([128, EC, 512], f32, tag="ft")
                    for oc in range(EC):
                        osl = slice(128 * oc, 128 * (oc + 1))
                        pf = psF.tile([128, 512], f32, tag="pf")
                        nc.tensor.matmul(pf[:], woT_sb[:, osl],
                                         ocat[:, ssl],
                                         start=True, stop=True)
                        nc.vector.tensor_copy(ft[:, oc, :], pf[:])
                    nc.sync.dma_start(out=out_r[:, :, ssl], in_=ft[:])

    nc.compile()
    return nc


def _classify_mask(mask):
    m = np.asarray(mask).reshape(S, S) != 0
    if m.all():
        return "none", None
    if np.array_equal(m, np.tril(np.ones((S, S), bool))):
        return "causal", None
    return "general", m.T.astype(np.float32)


def _ensure_ntff_hook():
    """Register antenv.axon_hooks with a ctypes NTFF profile hook if the
    container image lacks it (mirrors trn_agent_boot's registration)."""
    import types
    try:
        from antenv.axon_hooks import get_axon_ntff_profile_hook  # noqa: F401
        return
    except ImportError:
        pass
    import contextlib
    import ctypes

    hook = None
    so_path = "/opt/axon/libaxon_pjrt.so"
    try:
        lib = ctypes.CDLL(so_path)
        if hasattr(lib, "axon_start_nrt_profile"):
            lib.axon_start_nrt_profile.argtypes = [
                ctypes.POINTER(ctypes.c_int64), ctypes.c_size_t]
            lib.axon_start_nrt_profile.restype = ctypes.c_int64
            lib.axon_stop_nrt_profile.argtypes = [ctypes.c_char_p]
            lib.axon_stop_nrt_profile.restype = ctypes.c_int64

            @contextlib.contextmanager
            def _hook(output_dir, device_ids):
                import jax
                jax.devices()
                if device_ids:
                    ids = (ctypes.c_int64 * len(device_ids))(*device_ids)
                    rc = lib.axon_start_nrt_profile(ids, len(device_ids))
                else:
                    rc = lib.axon_start_nrt_profile(None, 0)
                if rc != 0:
                    raise RuntimeError(f"axon_start_nrt_profile rc={rc}")
                try:
                    yield
                finally:
                    n = lib.axon_stop_nrt_profile(str(output_dir).encode())
                    print(f"profile: {n} file(s) written to {output_dir}",
                          flush=True)

            hook = _hook
    except OSError:
        pass

    mod = types.ModuleType("antenv.axon_hooks")
    _h = [hook]
    mod.get_axon_ntff_profile_hook = lambda: _h[0]

    def _set(h):
        _h[0] = h

    mod.set_axon_ntff_profile_hook = _set
    sys.modules["antenv.axon_hooks"] = mod
    try:
        import antenv
        antenv.axon_hooks = mod
    except ImportError:
        pass


def kernel(key, query, value, mask, W_qkv, W_out, b_out):
    from concourse.bass_utils import run_bass_kernel_spmd
    import os

    mask_mode, maskT = _classify_mask(mask)
    if mask_mode not in _cache:
        _cache[mask_mode] = _build(mask_mode)
    nc = _cache[mask_mode]

    x = np.ascontiguousarray(
        np.asarray(query, np.float32).reshape(BS, D))
    xT_bf = np.ascontiguousarray(x.T).astype(BF16)
    W_qkv = np.asarray(W_qkv, np.float32)
    W_out = np.asarray(W_out, np.float32)

    in_maps = []
    for c in range(NCORES):
        sl = slice(SEC * c, SEC * (c + 1))
        wq = W_qkv[sl, :].T * np.float32(HD ** -0.5)
        wk = W_qkv[D + SEC * c:D + SEC * (c + 1), :].T
        wv = W_qkv[2 * D + SEC * c:2 * D + SEC * (c + 1), :].T
        consts = np.zeros((128, 160), BF16)
        consts[0:64, 0:64] = np.eye(64, dtype=BF16)
        consts[64:128, 0:64] = np.eye(64, dtype=BF16)
        consts[:, 64:160] = 1.0
        m = {
            "xT": xT_bf,
            "consts": consts,
            "wqkvT": np.ascontiguousarray(np.concatenate(
                [wq, wk, wv], axis=1, dtype=np.float32)).astype(BF16),
            "woT": np.ascontiguousarray(W_out[:, sl].T).astype(BF16),
        }
        if mask_mode == "general":
            m["maskT"] = maskT.astype(BF16)
        in_maps.append(m)

    trace = bool(int(os.environ.get("KERNEL_TRACE", "0")))
    if trace:
        _ensure_ntff_hook()
        try:
            res = run_bass_kernel_spmd(nc, in_maps,
                                       core_ids=list(range(NCORES)),
                                       trace=True)
        except Exception as e:
            print(f"traced run failed ({e!r}); retrying untraced",
                  flush=True)
            res = run_bass_kernel_spmd(nc, in_maps,
                                       core_ids=list(range(NCORES)))
        print(f"HW exec time: {res.exec_time_ns} ns", flush=True)
        kernel.last_exec_ns = res.exec_time_ns
        kernel.last_results = res
    else:
        res = run_bass_kernel_spmd(nc, in_maps, core_ids=list(range(NCORES)))

    acc = res.results[0]["out_pT"].astype(np.float32)
    for c in range(1, NCORES):
        acc = acc + res.results[c]["out_pT"]
    out = acc.T.reshape(B, S, D) + np.asarray(b_out, np.float32)
    return out.astype(np.float32)



# revision 4
# speedup vs baseline: 1.4920x; 1.4920x over previous
"""Multi-head attention forward, sharded head-parallel across 8 NeuronCores.

Per core c (heads 2c, 2c+1), all fp16:
  qT/kT/vT = (x @ W{q,k,v}_c.T).T       W.T-tiled matmuls vs xT, sequential
                                        q/k/v PSUM accumulation (2 banks)
  v1       = paired [128,128] PE transposes of vT (both heads at once),
             layout [kpos, KC, (v_h|1)*2] with interleaved ones columns
  scoresT  = kT_chunk.T @ qT            [k-pos partitions, q-pos free],
             diag blocks column-trimmed, ascending j so j=0 is full width
  probsT   = exp(scoresT), diag triangle zeroed by tri-mask multiply (any)
  av+rowsum: pos = [v_h | 1].T @ probsT (ones column yields softmax denom)
  normalize: reciprocal (DVE) -> partition_broadcast (gpsimd) -> mul (any)
  out_projT partial, casts spread via any-engine, fp16 partials out
Host: sum the 8 partial [1024, 4096] fp16 outputs in fp32, transpose, bias.
"""
import sys

sys.path.insert(0, "/opt/trn_rl_repo")

import ml_dtypes
import numpy as np

F16 = np.float16

B, S, D = 2, 2048, 1024
H, HD = 16, 64
NCORES = 8
SEC = 128           # output dims per core per section (2 heads * 64)
BS = B * S          # 4096
NT = BS // 512      # 8 seq tiles of 512
EC = D // 128       # 8 embed chunks
QT = S // 512       # 4 q-tiles per (b,h)
KC = S // 128       # 16 k-chunks per (b,h)

_cache = {}


def _build(mask_mode):
    import concourse.bass as bass
    import concourse.tile as tile
    from concourse import bacc, mybir

    f32 = mybir.dt.float32
    f16 = mybir.dt.float16
    Exp = mybir.ActivationFunctionType.Exp

    nc = bacc.Bacc("TRN2", target_bir_lowering=False, debug=False,
                   num_devices=NCORES)

    xT = nc.dram_tensor("xT", [D, BS], f16, kind="ExternalInput")
    wqkvT = nc.dram_tensor("wqkvT", [D, 3 * SEC], f16, kind="ExternalInput")
    woT = nc.dram_tensor("woT", [SEC, D], f16, kind="ExternalInput")
    # consts: [:, 0:128] = eye(128); [:, 128:384] = tri duplicated (2x128)
    consts = nc.dram_tensor("consts", [128, 384], f16, kind="ExternalInput")
    if mask_mode == "general":
        maskT = nc.dram_tensor("maskT", [S, S], f16, kind="ExternalInput")
    out_pT = nc.dram_tensor("out_pT", [D, BS], f16, kind="ExternalOutput")

    causal = mask_mode == "causal"

    with tile.TileContext(nc) as tc:
        with (
            nc.allow_low_precision(reason="fp16 attention pipeline"),
            tc.tile_pool(name="singles", bufs=1) as singles,
            tc.tile_pool(name="qkv", bufs=1) as qkv,
            tc.tile_pool(name="prp", bufs=4) as prp,
            tc.tile_pool(name="mskp", bufs=4) as mskp,
            tc.tile_pool(name="nrm", bufs=4) as nrm,
            tc.tile_pool(name="ftp", bufs=2) as ftp,
            tc.tile_pool(name="psA", bufs=2, space="PSUM") as psA,
            tc.tile_pool(name="psS", bufs=2, space="PSUM") as psS,
            tc.tile_pool(name="psO", bufs=2, space="PSUM") as psO,
        ):
            # ---- static loads (gpsimd queue) ----
            w_sb = singles.tile([128, EC, 3 * SEC], f16)
            wqr = wqkvT.rearrange("(ec p) c -> p ec c", p=128)
            for ec in range(EC):
                nc.gpsimd.dma_start(out=w_sb[:, ec, :], in_=wqr[:, ec, :])
            woT_sb = singles.tile([128, D], f16)
            nc.gpsimd.dma_start(out=woT_sb[:], in_=woT[:])
            ident = singles.tile([128, 128], f16)
            nc.gpsimd.dma_start(out=ident[:], in_=consts[:, 0:128])
            if causal:
                tri2 = singles.tile([128, 2, 128], f16)
                nc.gpsimd.dma_start(
                    out=tri2[:],
                    in_=consts[:, 128:384].rearrange("p (h c) -> p h c", h=2))

            # ---- x load, one DMA per 512-pos tile (sync queue) ----
            xfull = qkv.tile([128, EC, BS], f16)
            xTr = xT.rearrange("(ec p) s -> p ec s", p=128)
            for n in range(NT):
                sl = slice(512 * n, 512 * (n + 1))
                nc.sync.dma_start(out=xfull[:, :, sl], in_=xTr[:, :, sl])

            qT = qkv.tile([128, BS], f16)
            kT = qkv.tile([128, BS], f16)
            vT = qkv.tile([128, BS], f16)
            ocat = qkv.tile([128, BS], f16)
            v1s = []
            for b in range(B):
                v1 = qkv.tile([128, KC, 2, 65], f16, name=f"v1_{b}")
                v1s.append(v1)

            dsts = (qT, kT, vT)

            def stage_a(b):
                for n in range(4 * b, 4 * b + 4):
                    nsl = slice(512 * n, 512 * (n + 1))
                    for part in range(3):
                        pa = psA.tile([128, 512], f32, tag="pa", name="pa")
                        csl = slice(128 * part, 128 * (part + 1))
                        for ec in range(EC):
                            nc.tensor.matmul(pa[:], w_sb[:, ec, csl],
                                             xfull[:, ec, nsl],
                                             start=ec == 0, stop=ec == EC - 1)
                        nc.any.tensor_copy(dsts[part][:, nsl], pa[:])

            def stage_t(b):
                base = S * b
                v1 = v1s[b]
                nc.vector.memset(v1[:, :, :, 64], 1.0)
                for i in range(KC):
                    pt = psA.tile([128, 128], f16, tag="pa", name="pt")
                    nc.tensor.transpose(
                        pt[:], vT[:, base + 128 * i:base + 128 * (i + 1)],
                        ident[:])
                    nc.any.tensor_copy(
                        v1[:, i, :, 0:64],
                        pt.rearrange("p (h c) -> p h c", h=2))

            def stage_b(b):
                base = S * b
                v1 = v1s[b]
                for t in range(QT):
                    qsl = slice(base + 512 * t, base + 512 * (t + 1))
                    njc = 4 * t + 4 if causal else KC
                    pos = [psO.tile([65, 512], f32, tag="po",
                                    name=f"po{t}{lh}") for lh in range(2)]
                    for j in range(njc):
                        jm = j - 4 * t
                        c0 = 128 * jm if causal and jm >= 0 else 0
                        ksl = slice(base + 128 * j, base + 128 * (j + 1))
                        ps = psS.tile([128, 2, 512], f32, tag="ps", name="ps")
                        for lh in range(2):
                            hsl = slice(64 * lh, 64 * (lh + 1))
                            nc.tensor.matmul(
                                ps[:, lh, c0:], kT[hsl, ksl],
                                qT[hsl, qsl.start + c0:qsl.stop],
                                start=True, stop=True)
                        pr = prp.tile([128, 2, 512], f16, tag="pr", name="pr")
                        nc.scalar.activation(pr[:, :, c0:], ps[:, :, c0:], Exp)
                        if causal and jm >= 0:
                            win = slice(c0, c0 + 128)
                            nc.any.tensor_mul(pr[:, :, win], pr[:, :, win],
                                              tri2[:])
                        elif mask_mode == "general":
                            msk = mskp.tile([128, 512], f16, tag="mk",
                                            name="msk")
                            nc.sync.dma_start(
                                out=msk[:],
                                in_=maskT[128 * j:128 * (j + 1),
                                          512 * t:512 * (t + 1)])
                            for lh in range(2):
                                nc.any.tensor_mul(pr[:, lh, :], pr[:, lh, :],
                                                  msk[:])
                        for lh in range(2):
                            nc.tensor.matmul(
                                pos[lh][:, c0:], v1[:, j, lh, :],
                                pr[:, lh, c0:],
                                start=j == 0, stop=j == njc - 1)
                    for lh in range(2):
                        hsl = slice(64 * lh, 64 * (lh + 1))
                        # custom DVE ops read garbage from PSUM on HW --
                        # stage the denominator row through SBUF first
                        lc = nrm.tile([1, 512], f32, tag="lc", name="lc")
                        nc.vector.tensor_copy(lc[:], pos[lh][64:65, :])
                        rc = nrm.tile([1, 512], f32, tag="rc", name="rc")
                        nc.vector.reciprocal_approx_fast(rc[:], lc[:])
                        bs_ = nrm.tile([64, 512], f32, tag="bs", name="bs")
                        nc.gpsimd.partition_broadcast(bs_[:], rc[:])
                        nc.any.tensor_mul(ocat[hsl, qsl], pos[lh][0:64, :],
                                          bs_[:])

            out_r = out_pT.rearrange("(oc p) s -> p oc s", p=128)

            def stage_c(b):
                for n in range(4 * b, 4 * b + 4):
                    ssl = slice(512 * n, 512 * (n + 1))
                    ft = ftp.tile([128, EC, 512], f16, tag="ft", name="ft")
                    for oc in range(EC):
                        osl = slice(128 * oc, 128 * (oc + 1))
                        pf = psA.tile([128, 512], f32, tag="pa", name="pf")
                        nc.tensor.matmul(pf[:], woT_sb[:, osl],
                                         ocat[:, ssl], start=True, stop=True)
                        nc.any.tensor_copy(ft[:, oc, :], pf[:])
                    nc.sync.dma_start(out=out_r[:, :, ssl], in_=ft[:])

            # emission order chosen so the psA tag ring (A/T/C share it)
            # never makes batch-1 stage A wait on batch-0 stage C
            stage_a(0)
            stage_t(0)
            stage_b(0)
            stage_a(1)
            stage_t(1)
            stage_c(0)
            stage_b(1)
            stage_c(1)

    nc.compile()
    return nc


def _classify_mask(mask):
    m = np.asarray(mask).reshape(S, S) != 0
    if m.all():
        return "none", None
    if np.array_equal(m, np.tril(np.ones((S, S), bool))):
        return "causal", None
    return "general", m.T.astype(np.float32)


def _ensure_ntff_hook():
    """Register antenv.axon_hooks with a ctypes NTFF profile hook if the
    container image lacks it (mirrors trn_agent_boot's registration)."""
    import types
    try:
        from antenv.axon_hooks import get_axon_ntff_profile_hook  # noqa: F401
        return
    except ImportError:
        pass
    import contextlib
    import ctypes

    hook = None
    so_path = "/opt/axon/libaxon_pjrt.so"
    try:
        lib = ctypes.CDLL(so_path)
        if hasattr(lib, "axon_start_nrt_profile"):
            lib.axon_start_nrt_profile.argtypes = [
                ctypes.POINTER(ctypes.c_int64), ctypes.c_size_t]
            lib.axon_start_nrt_profile.restype = ctypes.c_int64
            lib.axon_stop_nrt_profile.argtypes = [ctypes.c_char_p]
            lib.axon_stop_nrt_profile.restype = ctypes.c_int64

            @contextlib.contextmanager
            def _hook(output_dir, device_ids):
                import jax
                jax.devices()
                if device_ids:
                    ids = (ctypes.c_int64 * len(device_ids))(*device_ids)
                    rc = lib.axon_start_nrt_profile(ids, len(device_ids))
                else:
                    rc = lib.axon_start_nrt_profile(None, 0)
                if rc != 0:
                    raise RuntimeError(f"axon_start_nrt_profile rc={rc}")
                try:
                    yield
                finally:
                    n = lib.axon_stop_nrt_profile(str(output_dir).encode())
                    print(f"profile: {n} file(s) written to {output_dir}",
                          flush=True)

            hook = _hook
    except OSError:
        pass

    mod = types.ModuleType("antenv.axon_hooks")
    _h = [hook]
    mod.get_axon_ntff_profile_hook = lambda: _h[0]

    def _set(h):
        _h[0] = h

    mod.set_axon_ntff_profile_hook = _set
    sys.modules["antenv.axon_hooks"] = mod
    try:
        import antenv
        antenv.axon_hooks = mod
    except ImportError:
        pass


def kernel(key, query, value, mask, W_qkv, W_out, b_out):
    from concourse.bass_utils import run_bass_kernel_spmd
    import os

    mask_mode, maskT = _classify_mask(mask)
    if mask_mode not in _cache:
        _cache[mask_mode] = _build(mask_mode)
    nc = _cache[mask_mode]

    x = np.ascontiguousarray(
        np.asarray(query, np.float32).reshape(BS, D))
    xT_f16 = np.ascontiguousarray(x.T).astype(F16)
    W_qkv = np.asarray(W_qkv, np.float32)
    W_out = np.asarray(W_out, np.float32)

    consts = np.zeros((128, 384), F16)
    consts[:, 0:128] = np.eye(128, dtype=F16)
    tri = (np.arange(128)[:, None] <= np.arange(128)[None, :]).astype(F16)
    consts[:, 128:256] = tri
    consts[:, 256:384] = tri

    in_maps = []
    for c in range(NCORES):
        sl = slice(SEC * c, SEC * (c + 1))
        wq = W_qkv[sl, :].T * np.float32(HD ** -0.5)
        wk = W_qkv[D + SEC * c:D + SEC * (c + 1), :].T
        wv = W_qkv[2 * D + SEC * c:2 * D + SEC * (c + 1), :].T
        m = {
            "xT": xT_f16,
            "consts": consts,
            "wqkvT": np.ascontiguousarray(np.concatenate(
                [wq, wk, wv], axis=1, dtype=np.float32)).astype(F16),
            "woT": np.ascontiguousarray(W_out[:, sl].T).astype(F16),
        }
        if mask_mode == "general":
            m["maskT"] = maskT.astype(F16)
        in_maps.append(m)

    trace = bool(int(os.environ.get("KERNEL_TRACE", "0")))
    if trace:
        _ensure_ntff_hook()
        try:
            res = run_bass_kernel_spmd(nc, in_maps,
                                       core_ids=list(range(NCORES)),
                                       trace=True)
        except Exception as e:
            print(f"traced run failed ({e!r}); retrying untraced",
                  flush=True)
            res = run_bass_kernel_spmd(nc, in_maps,
                                       core_ids=list(range(NCORES)))
        print(f"HW exec time: {res.exec_time_ns} ns", flush=True)
        kernel.last_exec_ns = res.exec_time_ns
        kernel.last_results = res
    else:
        res = run_bass_kernel_spmd(nc, in_maps, core_ids=list(range(NCORES)))
        kernel.last_results = res

    acc = res.results[0]["out_pT"].astype(np.float32)
    for c in range(1, NCORES):
        acc = acc + res.results[c]["out_pT"]
    out = acc.T.reshape(B, S, D) + np.asarray(b_out, np.float32)
    return out.astype(np.float32)
